# revision 30
# baseline (speedup 1.0000x reference)
"""Distributed Trainium2 Bass kernel for GQA causal attention
(S=2048, DIM=4096, NH=32, NKV=8, HD=128), tensor-parallel over heads on 8
NeuronCores.

Per-core program (core c owns q-heads 4c..4c+3 and kv-head c):
  1. QKV projection: qT/kT/vT = W.T-slices @ x.T   (bf16 matmul, f32 psum)
  2. RoPE on q/k via a signed pair-permutation matmul + DVE combine,
     output cast to bf16
  3. PE-transpose vT -> v (bf16)
  4. Causal attention in "scores-transposed" layout, all-bf16 matmuls:
     sT[kv,q] = kT.T q; exp on ACT (no max subtraction; scores are small);
     causal mask as a 0/1 bf16 multiply on diagonal blocks;
     denominator accumulated on the PE via a ones-matmul;
     yT[hd,q] += v.T p.
  5. Normalize (reciprocal computed in a [128,4] layout to use all DVE
     lanes), cast bf16, per-head AllGather -> full Y.T [4096, S]
  6. Output projection: core c computes out[:, 512c:512(c+1)] (as outT).

Host side shards/preps inputs (transposes, bf16 casts, cos/sin/mask/perm
tables) and concatenates the 8 output column-slices.
"""

import sys

sys.path.insert(0, "/opt/trn_rl_repo")

import numpy as np
import ml_dtypes

import concourse.bass as bass
import concourse.mybir as mybir
import concourse.tile as tile
from concourse import bacc
from concourse import bass_utils

S, DIM = 2048, 4096
NH, NKV, HD = 32, 8, 128
NCORES = 8
QH = NH // NCORES  # 4 q heads per core
KT = DIM // 128  # 32 contraction tiles
ST = S // 512  # 4 sequence tiles of 512
SCALE = 1.0 / float(np.sqrt(HD))

BF = mybir.dt.bfloat16
F32 = mybir.dt.float32
F32R = mybir.dt.float32r
ALU = mybir.AluOpType
ACTF = mybir.ActivationFunctionType


def r32(ap):
    return ap.bitcast(F32R)


def build_nc():
    nc = bacc.Bacc(
        "TRN2",
        target_bir_lowering=False,
        debug=False,
        enable_asserts=True,
        num_devices=NCORES,
    )

    xt = nc.dram_tensor("xt", [DIM, S], BF, kind="ExternalInput").ap()
    wqkvt = nc.dram_tensor("wqkvt", [DIM, 768], BF, kind="ExternalInput").ap()
    wot = nc.dram_tensor("wot", [DIM, 512], BF, kind="ExternalInput").ap()
    cost = nc.dram_tensor("cost", [128, S], F32, kind="ExternalInput").ap()
    sint = nc.dram_tensor("sint", [128, S], F32, kind="ExternalInput").ap()
    maskt = nc.dram_tensor("maskt", [128, 4, 512], BF, kind="ExternalInput").ap()
    rpermt = nc.dram_tensor("rpermt", [128, 128], F32R, kind="ExternalInput").ap()
    identt = nc.dram_tensor("identt", [128, 128], F32, kind="ExternalInput").ap()
    onest = nc.dram_tensor("onest", [128, 1], BF, kind="ExternalInput").ap()
    onescolt = nc.dram_tensor("onescolt", [1, 128], F32R, kind="ExternalInput").ap()
    outt = nc.dram_tensor("outt", [512, S], F32, kind="ExternalOutput").ap()

    with tile.TileContext(nc) as tc:
        with (
            tc.tile_pool(name="const", bufs=1) as const,
            tc.tile_pool(name="qkvsb", bufs=1) as qkvsb,
            tc.tile_pool(name="psacc", bufs=4, space="PSUM") as psacc,
            tc.tile_pool(name="psstr", bufs=2, space="PSUM") as psstr,
            tc.tile_pool(name="dram", bufs=1, space="DRAM") as dram,
        ):
            cos_sb = const.tile([128, S], F32)
            sin_sb = const.tile([128, S], F32)
            mask_sb = const.tile([128, 4, 512], BF)
            rperm_sb = const.tile([128, 128], F32R)
            ident_sb = const.tile([128, 128], F32)
            ones_sb = const.tile([128, 1], BF)
            onescol_sb = const.tile([1, 128], F32R)

            def load_consts():
                nc.sync.dma_start(cos_sb, cost)
                nc.sync.dma_start(sin_sb, sint)
                nc.sync.dma_start(mask_sb, maskt)
                nc.sync.dma_start(rperm_sb, rpermt)
                nc.sync.dma_start(ident_sb, identt)
                nc.sync.dma_start(ones_sb, onest)
                nc.sync.dma_start(onescol_sb, onescolt)

            # persistent activations, attention operands in bf16
            q_sb = qkvsb.tile([128, QH, S], BF)  # rope'd qT, head-major
            k_sb = qkvsb.tile([128, S], BF)  # rope'd kT
            # v, block-transposed, 129 cols per kv-block: [v(128) | ones]
            v_sb = qkvsb.tile([128, (S // 128) * 130], BF)

            # ---------------- phase 1: QKV projections + RoPE ----------------
            # two passes of 3 psums each over a resident x tile, so QKV only
            # needs 3 "acc" banks -- freeing 4 PSUM banks for score pairs
            with (
                tc.tile_pool(name="wqkv", bufs=1) as wqkv,
                tc.tile_pool(name="xs", bufs=2) as xs,
                tc.tile_pool(name="stg", bufs=3) as stg,
            ):
                w_sb = wqkv.tile([128, KT, 768], BF)
                wqkvt_r = wqkvt.rearrange("(kb p) m -> p kb m", p=128)

                def rope_tile(src_ps, dst_slice, s0):
                    """src_ps: [128,512] f32 psum (pre-rope). dst_slice: SBUF
                    bf16 [128,512] destination. s0: sequence offset."""
                    stage = stg.tile([128, 512], F32, tag="stage")
                    nc.vector.tensor_copy(r32(stage), src_ps)
                    rot = psacc.tile([128, 512], F32, tag="oacc", bufs=1)
                    nc.tensor.matmul(rot, rperm_sb, r32(stage))
                    t1 = stg.tile([128, 512], F32, tag="ropetmp")
                    nc.vector.tensor_tensor(
                        t1, stage, cos_sb[:, s0 : s0 + 512], ALU.mult
                    )
                    t2 = stg.tile([128, 512], F32, tag="ropetmp2")
                    nc.vector.tensor_tensor(
                        t2, rot, sin_sb[:, s0 : s0 + 512], ALU.mult
                    )
                    nc.vector.tensor_tensor(dst_slice, t1, t2, ALU.add)

                xt_r = xt.rearrange("(kb p) s -> p kb s", p=128)
                for si in range(ST):
                    s0 = 512 * si
                    xtile = xs.tile([128, KT, 512], BF, tag="xtile")
                    nc.sync.dma_start(
                        xtile[:, 0:16, :], xt_r[:, 0:16, s0 : s0 + 512]
                    )
                    nc.sync.dma_start(
                        xtile[:, 16:32, :], xt_r[:, 16:32, s0 : s0 + 512]
                    )
                    if si == 0:
                        for wc in range(4):
                            nc.sync.dma_start(
                                w_sb[:, 8 * wc : 8 * wc + 8, :],
                                wqkvt_r[:, 8 * wc : 8 * wc + 8, :],
                            )
                    for half, ms in [(0, (0, 1, 2)), (1, (3, 4, 5))]:
                        ps = {
                            m: psacc.tile(
                                [128, 512],
                                F32,
                                tag="acc",
                                bufs=3,
                                name=f"qkv_ps_{si}_{m}",
                            )
                            for m in ms
                        }
                        for k in range(KT):
                            for m in ms:
                                nc.tensor.matmul(
                                    ps[m],
                                    w_sb[:, k, 128 * m : 128 * (m + 1)],
                                    xtile[:, k, :],
                                    start=(k == 0),
                                    stop=(k == KT - 1),
                                )
                        if half == 0:
                            if si == 0:
                                load_consts()
                            for m in ms:
                                rope_tile(ps[m], q_sb[:, m, s0 : s0 + 512], s0)
                        else:
                            rope_tile(ps[3], q_sb[:, 3, s0 : s0 + 512], s0)
                            rope_tile(ps[4], k_sb[:, s0 : s0 + 512], s0)
                            # v: psum -> stage, then 4 PE transposes -> v_sb
                            vstage = stg.tile([128, 512], F32, tag="stage")
                            nc.vector.tensor_copy(vstage, ps[5])
                            for jj in range(4):
                                j = 4 * si + jj
                                vt_ps = psacc.tile(
                                    [128, 512], F32, tag="oacc", bufs=1
                                )
                                nc.tensor.transpose(
                                    vt_ps[:, 0:128],
                                    vstage[:, 128 * jj : 128 * (jj + 1)],
                                    ident_sb,
                                )
                                nc.vector.tensor_copy(
                                    v_sb[:, 130 * j : 130 * j + 128],
                                    vt_ps[:, 0:128],
                                )

            # ---------------- phases 3-5: attention, normalize, allgather ----
            # two AG halves: u=0 covers qt 0,1 (cols 0:512 / 512:1024),
            # u=1 covers qt 2,3 -- halves the per-collective fixed cost and
            # pipelines with out-proj consumption order
            y_bounce = {
                u: dram.tile(
                    [QH * 128, 1024], BF, tag=f"yb{u}", name=f"ybounce{u}"
                )
                for u in range(2)
            }
            y_gather = {
                u: dram.tile(
                    [NCORES * QH * 128, 1024],
                    BF,
                    addr_space="Shared",
                    tag=f"yg{u}",
                    name=f"ygather{u}",
                )
                for u in range(2)
            }

            with (
                tc.tile_pool(name="pp", bufs=6) as pp,
                tc.tile_pool(name="nrm", bufs=4) as nrm,
                tc.tile_pool(name="wo", bufs=1) as wo,
                tc.tile_pool(name="ys", bufs=4) as ys,
                tc.tile_pool(name="osb", bufs=4) as osb,
            ):
                # wo weights: DMA emitted mid-attention (at qt==2) so the
                # 4.2MB burst doesn't delay early normalize den/rec DMAs
                wo_sb = wo.tile([128, KT, 512], BF)
                wot_r = wot.rearrange("(kb p) m -> p kb m", p=128)

                def normalize(st):
                    # deferred epilogue: runs one (qt,h) group later so the
                    # reciprocal round-trip never head-of-line-blocks the PE
                    yraw, den_sb, nh, nqt = st
                    den_t = nrm.tile([128, 4], F32, tag="dent")
                    nc.sync.dma_start(den_t, den_sb)
                    rec_t = nrm.tile([128, 4], F32R, tag="rect")
                    with nc.allow_low_precision(reason="f32r for bcast matmul"):
                        nc.vector.reciprocal(rec_t, den_t)
                    rec_sb = nrm.tile([1, 512], F32R, tag="recsb")
                    nc.sync.dma_start(rec_sb, rec_t)
                    bc_ps = psacc.tile([128, 512], F32, tag="oacc", bufs=1)
                    nc.tensor.matmul(bc_ps, onescol_sb, rec_sb)
                    yn = nrm.tile([128, 512], BF, tag="yn")
                    nc.vector.tensor_tensor(yn, yraw, bc_ps, ALU.mult)
                    u, col = nqt // 2, 512 * (nqt % 2)
                    nc.sync.dma_start(
                        y_bounce[u][128 * nh : 128 * (nh + 1), col : col + 512],
                        yn,
                    )
                    if nh == QH - 1 and nqt % 2 == 1:
                        nc.gpsimd.collective_compute(
                            "AllGather",
                            ALU.bypass,
                            ins=[y_bounce[u].opt()],
                            outs=[y_gather[u].opt()],
                            replica_groups=[list(range(NCORES))],
                        )

                def emit_outproj_si(si, oc_groups, tags):
                    s0 = 512 * si
                    for ocs in oc_groups:
                        ops = {}
                        for oc in ocs:
                            ops[oc] = psacc.tile(
                                [128, 512],
                                F32,
                                tag=tags[oc],
                                bufs=3 if tags[oc] == "acc" else 1,
                                name=f"o_ps_{si}_{oc}",
                            )
                        yg_r = y_gather[si // 2].rearrange(
                            "(kb p) s -> p kb s", p=128
                        )
                        yc = 512 * (si % 2)
                        for k8 in range(KT // 8):
                            ytile = ys.tile(
                                [128, 8, 512], BF, tag="ytile", bufs=6
                            )
                            nc.scalar.dma_start(
                                ytile,
                                yg_r[:, 8 * k8 : 8 * k8 + 8, yc : yc + 512],
                            )
                            for kk in range(8):
                                k = 8 * k8 + kk
                                for oc in ocs:
                                    nc.tensor.matmul(
                                        ops[oc],
                                        wo_sb[:, k, 128 * oc : 128 * (oc + 1)],
                                        ytile[:, kk, :],
                                        start=(k == 0),
                                        stop=(k == KT - 1),
                                    )
                        for oc in ocs:
                            otile = osb.tile([128, 512], F32, tag="otile")
                            nc.vector.tensor_copy(otile, ops[oc])
                            nc.sync.dma_start(
                                outt[128 * oc : 128 * (oc + 1), s0 : s0 + 512],
                                otile,
                            )

                # tiny warmup collective: absorbs the first-collective
                # start latency on the cc stream while QKV still runs
                cc_wa = dram.tile([8, 16], BF, tag="ccwa", name="ccwa")
                cc_wb = dram.tile(
                    [64, 16], BF, addr_space="Shared", tag="ccwb", name="ccwb"
                )
                nc.gpsimd.collective_compute(
                    "AllGather",
                    ALU.bypass,
                    ins=[cc_wa.opt()],
                    outs=[cc_wb.opt()],
                    replica_groups=[list(range(NCORES))],
                )

                pending = []
                # forward qt order: small groups first, AGs trigger early and
                # the serial collective stream drains while attention runs
                for qt in range(ST):
                    if qt == 1:
                        nc.sync.dma_start(wo_sb[:, 0:16, :], wot_r[:, 0:16, :])
                        nc.sync.dma_start(wo_sb[:, 16:32, :], wot_r[:, 16:32, :])
                    for h in range(QH):
                        s0 = 512 * qt
                        nblocks = 4 * qt + 4
                        ya_ps = psacc.tile(
                            [128, 512], F32, tag="acc", bufs=3, name=f"ya_{h}_{qt}"
                        )
                        dn_ps = psacc.tile(
                            [128, 512], F32, tag="acc", bufs=3, name=f"dn_{h}_{qt}"
                        )

                        def emit_pair(pi):
                            """scores for blocks 2pi,2pi+1 into one flat
                            2-bank psum tile, ONE exp over both banks (halves
                            ACT instruction overhead; width trimmed to the
                            second block's triangular end), then causal masks
                            on diagonal blocks. Returns per-block (j,c0,N,p)."""
                            sT2 = psstr.tile([128, 1024], F32, tag="str", bufs=2)
                            meta = []
                            for t in range(2):
                                j = 2 * pi + t
                                jj = j - 4 * qt
                                c0 = 0 if jj < 0 else 128 * jj
                                N = 512 - c0
                                nc.tensor.matmul(
                                    sT2[:, 512 * t : 512 * t + N],
                                    k_sb[:, 128 * j : 128 * (j + 1)],
                                    q_sb[:, h, s0 + c0 : s0 + 512],
                                )
                                meta.append((j, jj, c0, N))
                            p2 = pp.tile([128, 1024], BF, tag="p")
                            W = 512 + meta[1][3]
                            nc.scalar.activation(
                                p2[:, :W], sT2[:, :W], ACTF.Exp, scale=SCALE
                            )
                            res = []
                            for t, (j, jj, c0, N) in enumerate(meta):
                                p_ap = p2[:, 512 * t : 512 * (t + 1)]
                                if jj >= 0:
                                    pm = pp.tile([128, 512], BF, tag="pm")
                                    nc.vector.tensor_tensor(
                                        pm[:, :N],
                                        p_ap[:, :N],
                                        mask_sb[:, jj, c0:512],
                                        ALU.mult,
                                    )
                                    p_ap = pm
                                res.append((j, c0, N, p_ap))
                            return res

                        # software pipeline at pair granularity: scores/exp
                        # of pair pi+1 in flight while pair pi's AV matmuls
                        # run on the PE
                        npairs = nblocks // 2
                        nd = 0
                        nxt = emit_pair(0)
                        for pi in range(npairs):
                            cur = nxt
                            if pi + 1 < npairs:
                                nxt = emit_pair(pi + 1)
                            for j, c0, N, p_ap in cur:
                                nc.tensor.matmul(
                                    ya_ps[:, c0:512],
                                    v_sb[:, 130 * j : 130 * j + 128],
                                    p_ap[:, :N],
                                    start=(j == 0),
                                    stop=(j == nblocks - 1),
                                )
                            if cur[1][0] - 4 * qt < 0:
                                # off-diagonal pair: one den matmul over the
                                # DVE-summed pair of p tiles
                                ps2 = pp.tile([128, 512], BF, tag="p2")
                                nc.vector.tensor_tensor(
                                    ps2, cur[0][3], cur[1][3], ALU.add
                                )
                                nc.tensor.matmul(
                                    dn_ps[0:1, :],
                                    ones_sb,
                                    ps2,
                                    start=(nd == 0),
                                    stop=False,
                                )
                                nd += 1
                            else:
                                for j, c0, N, p_ap in cur:
                                    nc.tensor.matmul(
                                        dn_ps[0:1, c0:512],
                                        ones_sb,
                                        p_ap[:, :N],
                                        start=(nd == 0),
                                        stop=(j == nblocks - 1),
                                    )
                                    nd += 1
                        # drain psum quickly; defer the rest
                        yraw = nrm.tile([128, 512], F32, tag="yraw")
                        nc.vector.tensor_copy(yraw, ya_ps)
                        den_sb = nrm.tile([1, 512], F32, tag="densb")
                        nc.vector.tensor_copy(den_sb, dn_ps[0:1, :])
                        if len(pending) >= 2:
                            normalize(pending.pop(0))
                        pending.append((yraw, den_sb, h, qt))
                    if qt == 1:
                        # flush so AG(qt 0-1) triggers now -- it heads the
                        # serial collective stream gating out-proj
                        while pending:
                            normalize(pending.pop(0))
                for st in pending:
                    normalize(st)
                for si in [0, 1, 2, 3]:
                    emit_outproj_si(
                        si,
                        [(0, 1, 2, 3)],
                        {0: "acc", 1: "acc", 2: "acc", 3: "oacc"},
                    )


    nc.compile()
    return nc


def make_in_maps(x, freqs_cis, wq, wk, wv, wo):
    f32 = np.float32
    bf = ml_dtypes.bfloat16
    xt = np.ascontiguousarray(x.T).astype(bf)
    cos = np.ascontiguousarray(np.repeat(freqs_cis[:, :, 0].T, 2, axis=0)).astype(f32)
    sin = np.ascontiguousarray(np.repeat(freqs_cis[:, :, 1].T, 2, axis=0)).astype(f32)
    kvi = np.arange(128, dtype=np.int64)[:, None]
    qi = np.arange(512, dtype=np.int64)[None, :]
    mask = np.stack(
        [(kvi + 128 * d <= qi).astype(f32) for d in range(4)], axis=1
    ).astype(bf)  # [128, 4, 512]
    rperm = np.zeros((128, 128), f32)
    for r in range(64):
        rperm[2 * r, 2 * r + 1] = -1.0
        rperm[2 * r + 1, 2 * r] = 1.0
    rpermT = np.ascontiguousarray(rperm.T)
    ident = np.eye(128, dtype=f32)
    ones = np.ones((128, 1), bf)
    onescol = np.ones((1, 128), f32)

    in_maps = []
    for c in range(NCORES):
        wqkv = np.concatenate(
            [
                wq[512 * c : 512 * (c + 1), :].T,
                wk[128 * c : 128 * (c + 1), :].T,
                wv[128 * c : 128 * (c + 1), :].T,
            ],
            axis=1,
        ).astype(bf)  # [DIM, 768]
        wot = np.ascontiguousarray(wo[512 * c : 512 * (c + 1), :].T).astype(bf)
        in_maps.append(
            {
                "xt": xt,
                "wqkvt": np.ascontiguousarray(wqkv),
                "wot": wot,
                "cost": cos,
                "sint": sin,
                "maskt": mask,
                "rpermt": rpermT,
                "identt": ident,
                "onest": ones,
                "onescolt": onescol,
            }
        )
    return in_maps


def install_ntff_hook():
    """Inject the missing ``antenv.axon_hooks`` module backed by ctypes calls
    into libaxon_pjrt.so, enabling run_bass_kernel_spmd(trace=True) under
    axon. Also neuter upload_artifacts (no artifact bucket here)."""
    import sys as _sys
    import types
    import ctypes
    import contextlib

    if "antenv.axon_hooks" in _sys.modules:
        return
    so_path = "/opt/axon/libaxon_pjrt.so"
    lib = ctypes.CDLL(so_path)
    lib.axon_start_nrt_profile.argtypes = [
        ctypes.POINTER(ctypes.c_int64),
        ctypes.c_size_t,
    ]
    lib.axon_start_nrt_profile.restype = ctypes.c_int64
    lib.axon_stop_nrt_profile.argtypes = [ctypes.c_char_p]
    lib.axon_stop_nrt_profile.restype = ctypes.c_int64

    @contextlib.contextmanager
    def _hook(output_dir, device_ids):
        import jax

        jax.devices()
        if device_ids:
            ids = (ctypes.c_int64 * len(device_ids))(*device_ids)
            rc = lib.axon_start_nrt_profile(ids, len(device_ids))
        else:
            rc = lib.axon_start_nrt_profile(None, 0)
        if rc != 0:
            raise RuntimeError(f"axon_start_nrt_profile rc={rc}")
        try:
            yield
        finally:
            n = lib.axon_stop_nrt_profile(str(output_dir).encode())
            print(f"ntff profile: {n} file(s) written to {output_dir}")

    mod = types.ModuleType("antenv.axon_hooks")
    mod.get_axon_ntff_profile_hook = lambda: _hook
    mod.set_axon_ntff_profile_hook = lambda h: None
    _sys.modules["antenv.axon_hooks"] = mod
    import antenv

    antenv.axon_hooks = mod
    bass_utils.upload_artifacts = lambda tmpdir: tmpdir


def run(x, freqs_cis, wq, wk, wv, wo, trace=False, trace_kwargs=None):
    if trace:
        install_ntff_hook()
    nc = build_nc()
    in_maps = make_in_maps(x, freqs_cis, wq, wk, wv, wo)
    res = bass_utils.run_bass_kernel_spmd(
        nc,
        in_maps,
        core_ids=list(range(NCORES)),
        trace=trace,
        **(trace_kwargs or {}),
    )
    outs = [r["outt"] for r in res.results]  # each [512, S] = outT slice
    full = np.concatenate([np.asarray(o).T for o in outs], axis=1).astype(np.float32)
    return full, res


def kernel(x, freqs_cis, wq, wk, wv, wo):
    full, _ = run(
        np.asarray(x, np.float32),
        np.asarray(freqs_cis, np.float32),
        np.asarray(wq, np.float32),
        np.asarray(wk, np.float32),
        np.asarray(wv, np.float32),
        np.asarray(wo, np.float32),
    )
    return full



# revision 31
# speedup vs baseline: 1.0227x; 1.0227x over previous
"""Distributed Trainium2 Bass kernel for GQA causal attention
(S=2048, DIM=4096, NH=32, NKV=8, HD=128), tensor-parallel over heads on 8
NeuronCores.

Per-core program (core c owns q-heads 4c..4c+3 and kv-head c):
  1. QKV projection: qT/kT/vT = W.T-slices @ x.T   (bf16 matmul, f32 psum)
  2. RoPE on q/k via a signed pair-permutation matmul + DVE combine,
     output cast to bf16
  3. PE-transpose vT -> v (bf16)
  4. Causal attention in "scores-transposed" layout, all-bf16 matmuls:
     sT[kv,q] = kT.T q; exp on ACT (no max subtraction; scores are small);
     causal mask as a 0/1 bf16 multiply on diagonal blocks;
     denominator accumulated on the PE via a ones-matmul;
     yT[hd,q] += v.T p.
  5. Normalize (reciprocal computed in a [128,4] layout to use all DVE
     lanes), cast bf16, per-head AllGather -> full Y.T [4096, S]
  6. Output projection: core c computes out[:, 512c:512(c+1)] (as outT).

Host side shards/preps inputs (transposes, bf16 casts, cos/sin/mask/perm
tables) and concatenates the 8 output column-slices.
"""

import sys

sys.path.insert(0, "/opt/trn_rl_repo")

import numpy as np
import ml_dtypes

import concourse.bass as bass
import concourse.mybir as mybir
import concourse.tile as tile
from concourse import bacc
from concourse import bass_utils

S, DIM = 2048, 4096
NH, NKV, HD = 32, 8, 128
NCORES = 8
QH = NH // NCORES  # 4 q heads per core
KT = DIM // 128  # 32 contraction tiles
ST = S // 512  # 4 sequence tiles of 512
SCALE = 1.0 / float(np.sqrt(HD))

BF = mybir.dt.bfloat16
F32 = mybir.dt.float32
F32R = mybir.dt.float32r
ALU = mybir.AluOpType
ACTF = mybir.ActivationFunctionType


def r32(ap):
    return ap.bitcast(F32R)


def build_nc():
    nc = bacc.Bacc(
        "TRN2",
        target_bir_lowering=False,
        debug=False,
        enable_asserts=True,
        num_devices=NCORES,
    )

    xt = nc.dram_tensor("xt", [DIM, S], BF, kind="ExternalInput").ap()
    wqkvt = nc.dram_tensor("wqkvt", [DIM, 768], BF, kind="ExternalInput").ap()
    wot = nc.dram_tensor("wot", [DIM, 512], BF, kind="ExternalInput").ap()
    cost = nc.dram_tensor("cost", [128, S], F32, kind="ExternalInput").ap()
    sint = nc.dram_tensor("sint", [128, S], F32, kind="ExternalInput").ap()
    maskt = nc.dram_tensor("maskt", [128, 4, 512], BF, kind="ExternalInput").ap()
    rpermt = nc.dram_tensor("rpermt", [128, 128], F32R, kind="ExternalInput").ap()
    identt = nc.dram_tensor("identt", [128, 128], F32, kind="ExternalInput").ap()
    onest = nc.dram_tensor("onest", [128, 1], BF, kind="ExternalInput").ap()
    onescolt = nc.dram_tensor("onescolt", [1, 128], F32R, kind="ExternalInput").ap()
    outt = nc.dram_tensor("outt", [512, S], F32, kind="ExternalOutput").ap()

    with tile.TileContext(nc) as tc:
        with (
            tc.tile_pool(name="const", bufs=1) as const,
            tc.tile_pool(name="qkvsb", bufs=1) as qkvsb,
            tc.tile_pool(name="psacc", bufs=4, space="PSUM") as psacc,
            tc.tile_pool(name="psstr", bufs=2, space="PSUM") as psstr,
            tc.tile_pool(name="dram", bufs=1, space="DRAM") as dram,
        ):
            cos_sb = const.tile([128, S], F32)
            sin_sb = const.tile([128, S], F32)
            mask_sb = const.tile([128, 4, 512], BF)
            rperm_sb = const.tile([128, 128], F32R)
            ident_sb = const.tile([128, 128], F32)
            ones_sb = const.tile([128, 1], BF)
            onescol_sb = const.tile([1, 128], F32R)

            def load_consts():
                nc.sync.dma_start(cos_sb, cost)
                nc.sync.dma_start(sin_sb, sint)
                nc.sync.dma_start(mask_sb, maskt)
                nc.sync.dma_start(rperm_sb, rpermt)
                nc.sync.dma_start(ident_sb, identt)
                nc.sync.dma_start(ones_sb, onest)
                nc.sync.dma_start(onescol_sb, onescolt)

            # persistent activations, attention operands in bf16
            q_sb = qkvsb.tile([128, QH, S], BF)  # rope'd qT, head-major
            k_sb = qkvsb.tile([128, S], BF)  # rope'd kT
            # v, block-transposed, 129 cols per kv-block: [v(128) | ones]
            v_sb = qkvsb.tile([128, (S // 128) * 130], BF)

            # ---------------- phase 1: QKV projections + RoPE ----------------
            # two passes of 3 psums each over a resident x tile, so QKV only
            # needs 3 "acc" banks -- freeing 4 PSUM banks for score pairs
            with (
                tc.tile_pool(name="wqkv", bufs=1) as wqkv,
                tc.tile_pool(name="xs", bufs=2) as xs,
                tc.tile_pool(name="stg", bufs=3) as stg,
            ):
                w_sb = wqkv.tile([128, KT, 768], BF)
                wqkvt_r = wqkvt.rearrange("(kb p) m -> p kb m", p=128)

                def rope_tile(src_ps, dst_slice, s0):
                    """src_ps: [128,512] f32 psum (pre-rope). dst_slice: SBUF
                    bf16 [128,512] destination. s0: sequence offset."""
                    stage = stg.tile([128, 512], F32, tag="stage")
                    nc.vector.tensor_copy(r32(stage), src_ps)
                    rot = psacc.tile([128, 512], F32, tag="oacc", bufs=1)
                    nc.tensor.matmul(rot, rperm_sb, r32(stage))
                    t1 = stg.tile([128, 512], F32, tag="ropetmp")
                    nc.vector.tensor_tensor(
                        t1, stage, cos_sb[:, s0 : s0 + 512], ALU.mult
                    )
                    t2 = stg.tile([128, 512], F32, tag="ropetmp2")
                    nc.vector.tensor_tensor(
                        t2, rot, sin_sb[:, s0 : s0 + 512], ALU.mult
                    )
                    nc.vector.tensor_tensor(dst_slice, t1, t2, ALU.add)

                xt_r = xt.rearrange("(kb p) s -> p kb s", p=128)
                for si in range(ST):
                    s0 = 512 * si
                    xtile = xs.tile([128, KT, 512], BF, tag="xtile")
                    nc.sync.dma_start(
                        xtile[:, 0:16, :], xt_r[:, 0:16, s0 : s0 + 512]
                    )
                    nc.sync.dma_start(
                        xtile[:, 16:32, :], xt_r[:, 16:32, s0 : s0 + 512]
                    )
                    if si == 0:
                        for wc in range(4):
                            nc.sync.dma_start(
                                w_sb[:, 8 * wc : 8 * wc + 8, :],
                                wqkvt_r[:, 8 * wc : 8 * wc + 8, :],
                            )
                    for half, ms in [(0, (0, 1, 2)), (1, (3, 4, 5))]:
                        ps = {
                            m: psacc.tile(
                                [128, 512],
                                F32,
                                tag="acc",
                                bufs=3,
                                name=f"qkv_ps_{si}_{m}",
                            )
                            for m in ms
                        }
                        for k in range(KT):
                            for m in ms:
                                nc.tensor.matmul(
                                    ps[m],
                                    w_sb[:, k, 128 * m : 128 * (m + 1)],
                                    xtile[:, k, :],
                                    start=(k == 0),
                                    stop=(k == KT - 1),
                                )
                        if half == 0:
                            if si == 0:
                                load_consts()
                            for m in ms:
                                rope_tile(ps[m], q_sb[:, m, s0 : s0 + 512], s0)
                        else:
                            rope_tile(ps[3], q_sb[:, 3, s0 : s0 + 512], s0)
                            rope_tile(ps[4], k_sb[:, s0 : s0 + 512], s0)
                            # v: psum -> stage, then 4 PE transposes -> v_sb
                            vstage = stg.tile([128, 512], F32, tag="stage")
                            nc.vector.tensor_copy(vstage, ps[5])
                            for jj in range(4):
                                j = 4 * si + jj
                                vt_ps = psacc.tile(
                                    [128, 512], F32, tag="oacc", bufs=1
                                )
                                nc.tensor.transpose(
                                    vt_ps[:, 0:128],
                                    vstage[:, 128 * jj : 128 * (jj + 1)],
                                    ident_sb,
                                )
                                nc.vector.tensor_copy(
                                    v_sb[:, 130 * j : 130 * j + 128],
                                    vt_ps[:, 0:128],
                                )

            # ---------------- phases 3-5: attention, normalize, allgather ----
            y_bounce = {
                qt: dram.tile(
                    [QH * 128, 512], BF, tag=f"yb{qt}", name=f"ybounce{qt}"
                )
                for qt in range(ST)
            }
            y_gather = {
                qt: dram.tile(
                    [NCORES * QH * 128, 512],
                    BF,
                    addr_space="Shared",
                    tag=f"yg{qt}",
                    name=f"ygather{qt}",
                )
                for qt in range(ST)
            }

            with (
                tc.tile_pool(name="pp", bufs=6) as pp,
                tc.tile_pool(name="nrm", bufs=4) as nrm,
                tc.tile_pool(name="wo", bufs=1) as wo,
                tc.tile_pool(name="ys", bufs=4) as ys,
                tc.tile_pool(name="osb", bufs=4) as osb,
            ):
                # wo weights: DMA emitted mid-attention (at qt==2) so the
                # 4.2MB burst doesn't delay early normalize den/rec DMAs
                wo_sb = wo.tile([128, KT, 512], BF)
                wot_r = wot.rearrange("(kb p) m -> p kb m", p=128)

                def normalize(st):
                    # deferred epilogue: runs one (qt,h) group later so the
                    # reciprocal round-trip never head-of-line-blocks the PE
                    yraw, den_sb, nh, nqt = st
                    den_t = nrm.tile([128, 4], F32, tag="dent")
                    nc.sync.dma_start(den_t, den_sb)
                    rec_t = nrm.tile([128, 4], F32R, tag="rect")
                    with nc.allow_low_precision(reason="f32r for bcast matmul"):
                        nc.vector.reciprocal(rec_t, den_t)
                    rec_sb = nrm.tile([1, 512], F32R, tag="recsb")
                    nc.sync.dma_start(rec_sb, rec_t)
                    bc_ps = psacc.tile([128, 512], F32, tag="oacc", bufs=1)
                    nc.tensor.matmul(bc_ps, onescol_sb, rec_sb)
                    yn = nrm.tile([128, 512], BF, tag="yn")
                    nc.vector.tensor_tensor(yn, yraw, bc_ps, ALU.mult)
                    nc.sync.dma_start(
                        y_bounce[nqt][128 * nh : 128 * (nh + 1), :], yn
                    )
                    if nh == QH - 1:
                        nc.gpsimd.collective_compute(
                            "AllGather",
                            ALU.bypass,
                            ins=[y_bounce[nqt].opt()],
                            outs=[y_gather[nqt].opt()],
                            replica_groups=[list(range(NCORES))],
                        )

                def emit_outproj_si(si, oc_groups, tags):
                    s0 = 512 * si
                    for ocs in oc_groups:
                        ops = {}
                        for oc in ocs:
                            ops[oc] = psacc.tile(
                                [128, 512],
                                F32,
                                tag=tags[oc],
                                bufs=3 if tags[oc] == "acc" else 1,
                                name=f"o_ps_{si}_{oc}",
                            )
                        yg_r = y_gather[si].rearrange("(kb p) s -> p kb s", p=128)
                        for k8 in range(KT // 8):
                            ytile = ys.tile(
                                [128, 8, 512], BF, tag="ytile", bufs=6
                            )
                            nc.scalar.dma_start(
                                ytile, yg_r[:, 8 * k8 : 8 * k8 + 8, :]
                            )
                            for kk in range(8):
                                k = 8 * k8 + kk
                                for oc in ocs:
                                    nc.tensor.matmul(
                                        ops[oc],
                                        wo_sb[:, k, 128 * oc : 128 * (oc + 1)],
                                        ytile[:, kk, :],
                                        start=(k == 0),
                                        stop=(k == KT - 1),
                                    )
                        for oc in ocs:
                            otile = osb.tile([128, 512], F32, tag="otile")
                            nc.vector.tensor_copy(otile, ops[oc])
                            nc.sync.dma_start(
                                outt[128 * oc : 128 * (oc + 1), s0 : s0 + 512],
                                otile,
                            )

                # tiny warmup collective: absorbs the first-collective
                # start latency on the cc stream while QKV still runs
                cc_wa = dram.tile([8, 16], BF, tag="ccwa", name="ccwa")
                cc_wb = dram.tile(
                    [64, 16], BF, addr_space="Shared", tag="ccwb", name="ccwb"
                )
                nc.gpsimd.collective_compute(
                    "AllGather",
                    ALU.bypass,
                    ins=[cc_wa.opt()],
                    outs=[cc_wb.opt()],
                    replica_groups=[list(range(NCORES))],
                )

                pending = []
                # forward qt order: small groups first, AGs trigger early and
                # the serial collective stream drains while attention runs
                for qt in range(ST):
                    if qt == 2:
                        nc.sync.dma_start(wo_sb[:, 0:16, :], wot_r[:, 0:16, :])
                        nc.sync.dma_start(wo_sb[:, 16:32, :], wot_r[:, 16:32, :])
                    for h in range(QH):
                        s0 = 512 * qt
                        nblocks = 4 * qt + 4
                        ya_ps = psacc.tile(
                            [128, 512], F32, tag="acc", bufs=3, name=f"ya_{h}_{qt}"
                        )
                        dn_ps = psacc.tile(
                            [128, 512], F32, tag="acc", bufs=3, name=f"dn_{h}_{qt}"
                        )

                        def emit_pair(pi):
                            """scores for blocks 2pi,2pi+1 into one flat
                            2-bank psum tile, ONE exp over both banks (halves
                            ACT instruction overhead; width trimmed to the
                            second block's triangular end), then causal masks
                            on diagonal blocks. Returns per-block (j,c0,N,p)."""
                            sT2 = psstr.tile(
                                [128, 2, 512], F32, tag="str", bufs=2
                            )
                            meta = []
                            for t in range(2):
                                j = 2 * pi + t
                                jj = j - 4 * qt
                                c0 = 0 if jj < 0 else 128 * jj
                                N = 512 - c0
                                nc.tensor.matmul(
                                    sT2[:, t, :N],
                                    k_sb[:, 128 * j : 128 * (j + 1)],
                                    q_sb[:, h, s0 + c0 : s0 + 512],
                                )
                                meta.append((j, jj, c0, N))
                            p2 = pp.tile([128, 2, 512], BF, tag="p")
                            nc.scalar.activation(p2, sT2, ACTF.Exp, scale=SCALE)
                            res = []
                            for t, (j, jj, c0, N) in enumerate(meta):
                                p_ap = p2[:, t, :]
                                if jj >= 0:
                                    pm = pp.tile([128, 512], BF, tag="pm")
                                    nc.vector.tensor_tensor(
                                        pm[:, :N],
                                        p_ap[:, :N],
                                        mask_sb[:, jj, c0:512],
                                        ALU.mult,
                                    )
                                    p_ap = pm
                                res.append((j, c0, N, p_ap))
                            return res

                        # software pipeline at pair granularity: scores/exp
                        # of pair pi+1 in flight while pair pi's AV matmuls
                        # run on the PE
                        npairs = nblocks // 2
                        nd = 0
                        nxt = emit_pair(0)
                        for pi in range(npairs):
                            cur = nxt
                            if pi + 1 < npairs:
                                nxt = emit_pair(pi + 1)
                            for j, c0, N, p_ap in cur:
                                nc.tensor.matmul(
                                    ya_ps[:, c0:512],
                                    v_sb[:, 130 * j : 130 * j + 128],
                                    p_ap[:, :N],
                                    start=(j == 0),
                                    stop=(j == nblocks - 1),
                                )
                            if cur[1][0] - 4 * qt < 0:
                                # off-diagonal pair: one den matmul over the
                                # DVE-summed pair of p tiles
                                ps2 = pp.tile([128, 512], BF, tag="p2")
                                nc.vector.tensor_tensor(
                                    ps2, cur[0][3], cur[1][3], ALU.add
                                )
                                nc.tensor.matmul(
                                    dn_ps[0:1, :],
                                    ones_sb,
                                    ps2,
                                    start=(nd == 0),
                                    stop=False,
                                )
                                nd += 1
                            else:
                                for j, c0, N, p_ap in cur:
                                    nc.tensor.matmul(
                                        dn_ps[0:1, c0:512],
                                        ones_sb,
                                        p_ap[:, :N],
                                        start=(nd == 0),
                                        stop=(j == nblocks - 1),
                                    )
                                    nd += 1
                        # drain psum quickly; defer the rest
                        yraw = nrm.tile([128, 512], F32, tag="yraw")
                        nc.vector.tensor_copy(yraw, ya_ps)
                        den_sb = nrm.tile([1, 512], F32, tag="densb")
                        nc.vector.tensor_copy(den_sb, dn_ps[0:1, :])
                        if len(pending) >= 2:
                            normalize(pending.pop(0))
                        pending.append((yraw, den_sb, h, qt))
                    if qt == 0:
                        # flush so AG(0) triggers now -- it heads the serial
                        # collective stream that gates all out-proj slices
                        while pending:
                            normalize(pending.pop(0))
                for st in pending:
                    normalize(st)
                for si in [0, 1, 2, 3]:
                    emit_outproj_si(
                        si,
                        [(0, 1, 2, 3)],
                        {0: "acc", 1: "acc", 2: "acc", 3: "oacc"},
                    )


    nc.compile()
    return nc


def make_in_maps(x, freqs_cis, wq, wk, wv, wo):
    f32 = np.float32
    bf = ml_dtypes.bfloat16
    xt = np.ascontiguousarray(x.T).astype(bf)
    cos = np.ascontiguousarray(np.repeat(freqs_cis[:, :, 0].T, 2, axis=0)).astype(f32)
    sin = np.ascontiguousarray(np.repeat(freqs_cis[:, :, 1].T, 2, axis=0)).astype(f32)
    kvi = np.arange(128, dtype=np.int64)[:, None]
    qi = np.arange(512, dtype=np.int64)[None, :]
    mask = np.stack(
        [(kvi + 128 * d <= qi).astype(f32) for d in range(4)], axis=1
    ).astype(bf)  # [128, 4, 512]
    rperm = np.zeros((128, 128), f32)
    for r in range(64):
        rperm[2 * r, 2 * r + 1] = -1.0
        rperm[2 * r + 1, 2 * r] = 1.0
    rpermT = np.ascontiguousarray(rperm.T)
    ident = np.eye(128, dtype=f32)
    ones = np.ones((128, 1), bf)
    onescol = np.ones((1, 128), f32)

    in_maps = []
    for c in range(NCORES):
        wqkv = np.concatenate(
            [
                wq[512 * c : 512 * (c + 1), :].T,
                wk[128 * c : 128 * (c + 1), :].T,
                wv[128 * c : 128 * (c + 1), :].T,
            ],
            axis=1,
        ).astype(bf)  # [DIM, 768]
        wot = np.ascontiguousarray(wo[512 * c : 512 * (c + 1), :].T).astype(bf)
        in_maps.append(
            {
                "xt": xt,
                "wqkvt": np.ascontiguousarray(wqkv),
                "wot": wot,
                "cost": cos,
                "sint": sin,
                "maskt": mask,
                "rpermt": rpermT,
                "identt": ident,
                "onest": ones,
                "onescolt": onescol,
            }
        )
    return in_maps


def install_ntff_hook():
    """Inject the missing ``antenv.axon_hooks`` module backed by ctypes calls
    into libaxon_pjrt.so, enabling run_bass_kernel_spmd(trace=True) under
    axon. Also neuter upload_artifacts (no artifact bucket here)."""
    import sys as _sys
    import types
    import ctypes
    import contextlib

    if "antenv.axon_hooks" in _sys.modules:
        return
    so_path = "/opt/axon/libaxon_pjrt.so"
    lib = ctypes.CDLL(so_path)
    lib.axon_start_nrt_profile.argtypes = [
        ctypes.POINTER(ctypes.c_int64),
        ctypes.c_size_t,
    ]
    lib.axon_start_nrt_profile.restype = ctypes.c_int64
    lib.axon_stop_nrt_profile.argtypes = [ctypes.c_char_p]
    lib.axon_stop_nrt_profile.restype = ctypes.c_int64

    @contextlib.contextmanager
    def _hook(output_dir, device_ids):
        import jax

        jax.devices()
        if device_ids:
            ids = (ctypes.c_int64 * len(device_ids))(*device_ids)
            rc = lib.axon_start_nrt_profile(ids, len(device_ids))
        else:
            rc = lib.axon_start_nrt_profile(None, 0)
        if rc != 0:
            raise RuntimeError(f"axon_start_nrt_profile rc={rc}")
        try:
            yield
        finally:
            n = lib.axon_stop_nrt_profile(str(output_dir).encode())
            print(f"ntff profile: {n} file(s) written to {output_dir}")

    mod = types.ModuleType("antenv.axon_hooks")
    mod.get_axon_ntff_profile_hook = lambda: _hook
    mod.set_axon_ntff_profile_hook = lambda h: None
    _sys.modules["antenv.axon_hooks"] = mod
    import antenv

    antenv.axon_hooks = mod
    bass_utils.upload_artifacts = lambda tmpdir: tmpdir


def run(x, freqs_cis, wq, wk, wv, wo, trace=False, trace_kwargs=None):
    if trace:
        install_ntff_hook()
    nc = build_nc()
    in_maps = make_in_maps(x, freqs_cis, wq, wk, wv, wo)
    res = bass_utils.run_bass_kernel_spmd(
        nc,
        in_maps,
        core_ids=list(range(NCORES)),
        trace=trace,
        **(trace_kwargs or {}),
    )
    outs = [r["outt"] for r in res.results]  # each [512, S] = outT slice
    full = np.concatenate([np.asarray(o).T for o in outs], axis=1).astype(np.float32)
    return full, res


def kernel(x, freqs_cis, wq, wk, wv, wo):
    full, _ = run(
        np.asarray(x, np.float32),
        np.asarray(freqs_cis, np.float32),
        np.asarray(wq, np.float32),
        np.asarray(wk, np.float32),
        np.asarray(wv, np.float32),
        np.asarray(wo, np.float32),
    )
    return full

